# revision 18
# baseline (speedup 1.0000x reference)
"""MoE routing kernel for Trainium2 (8 NeuronCores, SPMD expert-parallel).

Contract: kernel(**full_inputs) -> full output [B, S, H] float32.

Strategy
--------
- Host: compute the (tiny) gate + group-topk routing in numpy (bit-identical
  selection to the jax reference), build per-(token,expert) combine weights,
  and dispatch: gather each expert's tokens into padded, transposed bf16
  buffers (the "all-to-all by topk_idx" of the sharding hint, done at
  input-sharding time).
- Device (SPMD over 8 cores):
    * Shared expert is TOKEN-sharded: each core runs the full SwiGLU
      (SI = 2816 = 22x128, zero padding) over its own T/8 = 512 tokens.
    * Routed experts are EXPERT-sharded with load balancing: each expert's
      token list is split into two halves; the 32 half-pieces are ranked by
      size and placed into 4 fixed-capacity slots x 8 cores so every core
      gets an equal, near-minimal amount of matmul columns.  Each (core,
      slot) holds one half-expert: its gathered tokens + that expert's
      gate/up/down weight panels.
- Host: scale per-piece outputs by routing weights, scatter-add, place the
  per-core shared slices, transpose back.

All matmuls run in bf16 with fp32 PSUM accumulation.  Weight panels are
pre-tiled on the host into the exact SBUF tile layout so each streams from
HBM once as a contiguous per-partition DMA; token tiles stay k-resident;
weights load on the sync queue, x tiles on scalar/gpsimd, outputs on
gpsimd, so no queue serializes against another's burst.  The very first
weight tiles are split into quarter-DMAs so the PE starts within ~1us.
"""

import math

import numpy as np
import ml_dtypes

H = 2048          # hidden size
I = 1408          # intermediate per routed expert
E = 16            # routed experts
G = 4             # groups
TOPK_GROUP = 2
TOP_K = 6
N_SHARED = 2
SCALE_FACTOR = 2.5
SI = I * N_SHARED   # 2816 shared intermediate
N_CORES = 8
NSLOT = 4           # routed half-expert slots per core
P = 128
KH = H // P         # 16 contraction chunks over H
MI = I // P         # 11 I chunks (routed)
MSI = SI // P       # 22 I chunks (shared)
MH = H // P         # 16 output H chunks
TS = 4096 // N_CORES  # 512 tokens per core for the shared expert
BF16 = ml_dtypes.bfloat16

_COMPILED = {}  # caps tuple -> nc
_LAST = {}      # debug/profiling handle for test.py


def _gate_host(hs, gate_weight, bias):
    """numpy replica of reference._gate (verified bit-identical selection)."""
    T = hs.shape[0]
    logits = hs @ gate_weight.T                       # [T, E] fp32
    scores = 1.0 / (1.0 + np.exp(-logits))
    sfc = scores + bias[None, :]
    gs = sfc.reshape(T, G, E // G)
    gsort = np.sort(gs, axis=-1)
    group_scores = gsort[..., -1] + gsort[..., -2]
    group_idx = np.argsort(-group_scores, axis=-1, kind="stable")[:, :TOPK_GROUP]
    gmask = np.zeros((T, G), bool)
    gmask[np.arange(T)[:, None], group_idx] = True
    smask = np.repeat(gmask, E // G, axis=1)
    tmp = np.where(smask, sfc, 0.0)
    topk_idx = np.argsort(-tmp, axis=-1, kind="stable")[:, :TOP_K]
    topk_w = np.take_along_axis(scores, topk_idx, axis=1)
    topk_w = topk_w / (topk_w.sum(-1, keepdims=True) + 1e-20) * SCALE_FACTOR
    return topk_idx.astype(np.int32), topk_w.astype(np.float32)


def _build(caps):
    """Build + compile the SPMD Bass program.

    caps : per routed slot capacity (tokens, each = 2*w with w % 4 == 0)
    """
    import concourse.mybir as mybir
    import concourse.tile as tile
    from concourse import bacc

    bf = mybir.dt.bfloat16
    f32 = mybir.dt.float32
    AF = mybir.ActivationFunctionType

    C_tot = sum(caps)
    slot_base = [sum(caps[:s]) for s in range(len(caps))]
    ws = []
    for cap in caps:
        assert cap % 8 == 0
        ws.append(cap // 2)

    nc = bacc.Bacc("TRN2", target_bir_lowering=False, debug=False,
                   num_devices=N_CORES)
    xs = nc.dram_tensor("xs", [H, TS], bf, kind="ExternalInput")
    xg = nc.dram_tensor("xg", [H, C_tot], bf, kind="ExternalInput")
    # weight panels are pre-tiled on the host to the exact SBUF tile layout
    # [tile_idx, partition, ko*128+c] so every load is a contiguous
    # per-partition stream
    wg = nc.dram_tensor("wg", [NSLOT * MI, P, KH * P], bf,
                        kind="ExternalInput")
    wu = nc.dram_tensor("wu", [NSLOT * MI, P, KH * P], bf,
                        kind="ExternalInput")
    wd = nc.dram_tensor("wd", [NSLOT * MH, P, MI * P], bf,
                        kind="ExternalInput")
    sg = nc.dram_tensor("sg", [MSI, P, KH * P], bf, kind="ExternalInput")
    su = nc.dram_tensor("su", [MSI, P, KH * P], bf, kind="ExternalInput")
    sd = nc.dram_tensor("sd", [MH, P, MSI * P], bf, kind="ExternalInput")
    ye = nc.dram_tensor("ye", [H, C_tot], bf, kind="ExternalOutput")
    ys = nc.dram_tensor("ys", [H, TS], bf, kind="ExternalOutput")

    with tile.TileContext(nc) as tc:
        with (
            tc.tile_pool(name="xp", bufs=34) as xp,    # x tiles [128, <=1024]
            tc.tile_pool(name="wp", bufs=8) as wp,     # [128,16,128] g/u cols
            tc.tile_pool(name="wq", bufs=8) as wqp,    # [128,4,128] m0 quarters
            tc.tile_pool(name="wdp", bufs=4) as wdp,   # [128,11,128] down
            tc.tile_pool(name="sdp", bufs=3) as sdp,   # [128,22,128] sh down
            tc.tile_pool(name="itp", bufs=27) as itp,  # [128,512] bf16 inter
            tc.tile_pool(name="tmp", bufs=4) as tmp,   # silu temp
            tc.tile_pool(name="otp", bufs=6) as otp,   # [128,<=1024] bf16 out
            tc.tile_pool(name="pg", bufs=2, space="PSUM") as pgp,
            tc.tile_pool(name="pu", bufs=2, space="PSUM") as pup,
            tc.tile_pool(name="py", bufs=4, space="PSUM") as pyp,
        ):
            # ---------------- section order: routed slots 0..3, shared last.
            # Routed gate/up chains consume weights at ~94 GB/s (each tile
            # feeds 2*w columns) vs ~147 GB/s for shared, so starting with
            # a routed slot lets the DMA pipeline fill without stalling the
            # PE.  Each section's x tiles are drip-fed from inside the
            # previous section's down loop (one chunk per M iteration) so
            # the in-order scalar queue interleaves outputs and prefetch.
            def load_gu(pool, tagname, dram, row, nm, split):
                if split:
                    parts = []
                    for q in range(4):
                        t = wqp.tile([P, 4, P], bf, name=f"{nm}_{q}",
                                     tag="wq")
                        nc.sync.dma_start(
                            t[:],
                            dram[row].rearrange(
                                "p (ko c) -> p ko c", c=P)[:, 4 * q:4 * q + 4, :])
                        parts.append(t)
                    return parts
                t = pool.tile([P, KH, P], bf, name=nm, tag=tagname)
                nc.sync.dma_start(
                    t[:], dram[row].rearrange("p (ko c) -> p ko c", c=P))
                return t

            def gu_slice(t, k):
                if isinstance(t, list):
                    return t[k // 4][:, k % 4, :]
                return t[:, k, :]

            xg_tiles = {}  # (s, k) -> x tile

            def emit_xg(s, k, eng=None):
                cap = caps[s]
                t = xp.tile([P, cap], bf, name=f"xg{s}_{k}", tag="x")
                (eng or nc.scalar).dma_start(
                    t[:], xg[k * P:(k + 1) * P,
                             slot_base[s]:slot_base[s] + cap])
                xg_tiles[(s, k)] = t

            xs_tiles = []

            def emit_xs(k):
                t = xp.tile([P, TS], bf, name=f"xs{k}", tag="x")
                nc.gpsimd.dma_start(t[:], xs[k * P:(k + 1) * P, :])
                xs_tiles.append(t)

            def emit_routed(s, first, prefetch):
                w = ws[s]
                cap = caps[s]
                b0 = slot_base[s]
                xgt = [xg_tiles[(s, k)] for k in range(KH)]
                inter = {}
                for m in range(MI):
                    split = first and m == 0
                    if split:
                        wgt, wut = first  # preloaded interleaved at top
                    else:
                        wgt = load_gu(wp, "wp", wg, s * MI + m,
                                      f"wgt{s}_{m}", False)
                        wut = load_gu(wp, "wp", wu, s * MI + m,
                                      f"wut{s}_{m}", False)
                    if split:
                        # k-outer warmup: each arriving x chunk feeds 4
                        # matmuls (gate/up x j0/j1) so the PE keeps pace
                        # with the DMA feed during the first pass.
                        pst = {}
                        for gu in range(2):
                            pool_, wt = [(pgp, "pg"), (pup, "pu")][gu]
                            for j in range(2):
                                pst[(gu, j)] = pool_.tile(
                                    [P, TS], f32, name=f"pw{s}{gu}{j}",
                                    tag=wt)
                        for k in range(KH):
                            for gu, wtile in ((0, wgt), (1, wut)):
                                for j in range(2):
                                    nc.tensor.matmul(
                                        pst[(gu, j)][:, :w],
                                        gu_slice(wtile, k),
                                        xgt[k][:, j * w:(j + 1) * w],
                                        start=(k == 0), stop=(k == KH - 1))
                        for j in range(2):
                            st = tmp.tile([P, TS], bf, name=f"st{s}_0{j}",
                                          tag="tmp")
                            nc.scalar.activation(st[:, :w],
                                                 pst[(0, j)][:, :w], AF.Silu)
                            it = itp.tile([P, TS], bf, name=f"it{s}_0{j}",
                                          tag="it")
                            nc.vector.tensor_mul(it[:, :w], st[:, :w],
                                                 pst[(1, j)][:, :w])
                            inter[(0, j)] = it
                        continue
                    for j in range(2):
                        psg = pgp.tile([P, TS], f32, name=f"psg{s}_{m}{j}",
                                       tag="pg")
                        for k in range(KH):
                            nc.tensor.matmul(
                                psg[:, :w], gu_slice(wgt, k),
                                xgt[k][:, j * w:(j + 1) * w],
                                start=(k == 0), stop=(k == KH - 1))
                        st = tmp.tile([P, TS], bf, name=f"st{s}_{m}{j}",
                                      tag="tmp")
                        nc.scalar.activation(st[:, :w], psg[:, :w], AF.Silu)
                        psu = pup.tile([P, TS], f32, name=f"psu{s}_{m}{j}",
                                       tag="pu")
                        for k in range(KH):
                            nc.tensor.matmul(
                                psu[:, :w], gu_slice(wut, k),
                                xgt[k][:, j * w:(j + 1) * w],
                                start=(k == 0), stop=(k == KH - 1))
                        it = itp.tile([P, TS], bf, name=f"it{s}_{m}{j}",
                                      tag="it")
                        nc.vector.tensor_mul(it[:, :w], st[:, :w],
                                             psu[:, :w])
                        inter[(m, j)] = it
                for M in range(MH):
                    wdt = wdp.tile([P, MI, P], bf, name=f"wdt{s}_{M}",
                                   tag="wdt")
                    nc.sync.dma_start(
                        wdt[:], wd[s * MH + M].rearrange(
                            "p (ko c) -> p ko c", c=P))
                    ot = otp.tile([P, 2 * w], bf, name=f"ot{s}_{M}", tag="ot")
                    for j in range(2):
                        psy = pyp.tile([P, TS], f32, name=f"psy{s}_{M}{j}",
                                       tag="py")
                        for K in range(MI):
                            nc.tensor.matmul(
                                psy[:, :w], wdt[:, K, :],
                                inter[(K, j)][:, :w],
                                start=(K == 0), stop=(K == MI - 1))
                        nc.vector.tensor_copy(
                            ot[:, j * w:(j + 1) * w], psy[:, :w])
                        nc.scalar.dma_start(
                            ye[M * P:(M + 1) * P,
                               b0 + j * w:b0 + (j + 1) * w],
                            ot[:, j * w:(j + 1) * w])
                    prefetch(M)

            def emit_shared():
                sint = []
                for m in range(MSI):
                    sgt = load_gu(wp, "wp", sg, m, f"sgt{m}", False)
                    sut = load_gu(wp, "wp", su, m, f"sut{m}", False)
                    psg = pgp.tile([P, TS], f32, name=f"psgs{m}", tag="pg")
                    for k in range(KH):
                        nc.tensor.matmul(psg[:], sgt[:, k, :], xs_tiles[k][:],
                                         start=(k == 0), stop=(k == KH - 1))
                    st = tmp.tile([P, TS], bf, name=f"sts{m}", tag="tmp")
                    nc.scalar.activation(st[:], psg[:], AF.Silu)
                    psu = pup.tile([P, TS], f32, name=f"psus{m}", tag="pu")
                    for k in range(KH):
                        nc.tensor.matmul(psu[:], sut[:, k, :], xs_tiles[k][:],
                                         start=(k == 0), stop=(k == KH - 1))
                    it = itp.tile([P, TS], bf, name=f"si{m}", tag="it")
                    nc.vector.tensor_mul(it[:], st[:], psu[:])
                    sint.append(it)
                for M in range(MH):
                    sdt = sdp.tile([P, MSI, P], bf, name=f"sdt{M}", tag="sdt")
                    nc.sync.dma_start(
                        sdt[:], sd[M].rearrange("p (ko c) -> p ko c", c=P))
                    psy = pyp.tile([P, TS], f32, name=f"psys{M}", tag="py")
                    for K in range(MSI):
                        nc.tensor.matmul(psy[:], sdt[:, K, :], sint[K][:],
                                         start=(K == 0), stop=(K == MSI - 1))
                    ot = otp.tile([P, TS], bf, name=f"ots{M}", tag="ot")
                    nc.vector.tensor_copy(ot[:], psy[:])
                    nc.scalar.dma_start(ys[M * P:(M + 1) * P, :], ot[:])

            # slot0 startup: interleave the m0 weight quarters and the odd
            # xg chunks on the sync queue in PE consumption order; even xg
            # chunks stream on scalar in parallel.
            wgt0, wut0 = [], []
            for q in range(4):
                for dram, lst, nm in ((wg, wgt0, "wgt0"), (wu, wut0, "wut0")):
                    t = wqp.tile([P, 4, P], bf, name=f"{nm}_{q}", tag="wq")
                    nc.sync.dma_start(
                        t[:],
                        dram[0].rearrange(
                            "p (ko c) -> p ko c", c=P)[:, 4 * q:4 * q + 4, :])
                    lst.append(t)
                emit_xg(0, 4 * q + 1, nc.sync)
                emit_xg(0, 4 * q + 3, nc.sync)
            for k in range(0, KH, 2):
                emit_xg(0, k)
            emit_routed(0, (wgt0, wut0), lambda M: emit_xg(1, M))
            emit_routed(1, False, lambda M: emit_xg(2, M))
            emit_routed(2, False, lambda M: emit_xg(3, M))
            emit_routed(3, False, lambda M: emit_xs(M))
            emit_shared()

    nc.compile()
    return nc


def _get_compiled(caps):
    key = tuple(caps)
    if key not in _COMPILED:
        _COMPILED[key] = _build(caps)
    return _COMPILED[key]


def _tile_gu(wmat, mi):  # [mi*P, H] -> [mi, P, KH*P] : (m, c_i, ko_h*P + p_i)
    return np.ascontiguousarray(
        wmat.reshape(mi, P, KH, P).transpose(0, 3, 2, 1)
    ).reshape(mi, P, KH * P).astype(BF16)


def _tile_dn(wmat, mi):  # [H, mi*P] -> [MH, P, mi*P]
    return np.ascontiguousarray(
        wmat.reshape(MH, P, mi, P).transpose(0, 3, 2, 1)
    ).reshape(MH, P, mi * P).astype(BF16)


def kernel(hidden_states, gate_weight, e_score_correction_bias,
           gate_proj, up_proj, down_proj,
           shared_gate_w, shared_up_w, shared_down_w):
    from concourse.bass_utils import run_bass_kernel_spmd

    hs = np.asarray(hidden_states, dtype=np.float32)
    B, S, Hh = hs.shape
    assert Hh == H
    hsf = np.ascontiguousarray(hs.reshape(-1, H))
    T = hsf.shape[0]
    assert T == N_CORES * TS
    gate_weight = np.asarray(gate_weight, np.float32)
    bias = np.asarray(e_score_correction_bias, np.float32)
    gate_proj = np.asarray(gate_proj, np.float32)
    up_proj = np.asarray(up_proj, np.float32)
    down_proj = np.asarray(down_proj, np.float32)
    shared_gate_w = np.asarray(shared_gate_w, np.float32)
    shared_up_w = np.asarray(shared_up_w, np.float32)
    shared_down_w = np.asarray(shared_down_w, np.float32)

    # ---- routing on host ----
    topk_idx, topk_w = _gate_host(hsf, gate_weight, bias)
    comb = np.zeros((T, E), np.float32)
    np.add.at(comb, (np.arange(T)[:, None], topk_idx), topk_w)
    sel = np.zeros((T, E), bool)
    sel[np.arange(T)[:, None], topk_idx] = True
    idx_e = [np.nonzero(sel[:, e])[0] for e in range(E)]

    # split each expert's tokens in half; rank the 32 pieces by size and
    # deal them into NSLOT slots of 8 cores so per-slot capacity (uniform
    # across cores under SPMD) hugs the max piece of its rank
    pieces = []
    for e in range(E):
        n = len(idx_e[e])
        h1 = (n + 1) // 2
        pieces.append((e, idx_e[e][:h1]))
        pieces.append((e, idx_e[e][h1:]))
    pieces.sort(key=lambda p: -len(p[1]))
    assert len(pieces) == NSLOT * N_CORES
    caps, assign = [], []
    for r in range(NSLOT):
        grp = pieces[N_CORES * r:N_CORES * (r + 1)]
        cap = max(8, -(-max(len(p[1]) for p in grp) // 8) * 8)
        caps.append(cap)
        assign.append(grp)
    slot_base = [sum(caps[:s]) for s in range(NSLOT)]
    C_tot = sum(caps)

    # ---- host-side dispatch (shard + transpose + bf16 cast) ----
    xsT = np.ascontiguousarray(hsf.T).astype(BF16)          # [H, T]

    gu_g = {e: _tile_gu(gate_proj[e], MI) for e in range(E)}
    gu_u = {e: _tile_gu(up_proj[e], MI) for e in range(E)}
    dn = {e: _tile_dn(down_proj[e], MI) for e in range(E)}
    sg_p = _tile_gu(shared_gate_w, MSI)
    su_p = _tile_gu(shared_up_w, MSI)
    sd_p = _tile_dn(shared_down_w, MSI)

    in_maps = []
    for c in range(N_CORES):
        xg_c = np.zeros((H, C_tot), BF16)
        for r in range(NSLOT):
            e, idx = assign[r][c]
            xg_c[:, slot_base[r]:slot_base[r] + len(idx)] = xsT[:, idx]
        wg_c = np.concatenate([gu_g[assign[r][c][0]] for r in range(NSLOT)])
        wu_c = np.concatenate([gu_u[assign[r][c][0]] for r in range(NSLOT)])
        wd_c = np.concatenate([dn[assign[r][c][0]] for r in range(NSLOT)])
        xs_c = np.ascontiguousarray(xsT[:, c * TS:(c + 1) * TS])
        in_maps.append({
            "xs": xs_c, "xg": xg_c,
            "wg": wg_c, "wu": wu_c, "wd": wd_c,
            "sg": sg_p, "su": su_p, "sd": sd_p,
        })

    nc = _get_compiled(caps)
    results = run_bass_kernel_spmd(nc, in_maps, core_ids=list(range(N_CORES)))

    _LAST.clear()
    _LAST.update(nc=nc, in_maps=in_maps, results=results, caps=caps,
                 assign=assign)

    # ---- host-side combine ----
    outT = np.empty((H, T), np.float32)
    for c in range(N_CORES):
        outT[:, c * TS:(c + 1) * TS] = results.results[c]["ys"]
    for c in range(N_CORES):
        ye = results.results[c]["ye"].astype(np.float32)
        for r in range(NSLOT):
            e, idx = assign[r][c]
            if len(idx) == 0:
                continue
            b0 = slot_base[r]
            we = comb[idx, e]
            outT[:, idx] += ye[:, b0:b0 + len(idx)] * we[None, :]

    return np.ascontiguousarray(outT.T).reshape(B, S, H).astype(np.float32)


# revision 20
# speedup vs baseline: 1.0128x; 1.0128x over previous
"""MoE routing kernel for Trainium2 (8 NeuronCores, SPMD expert-parallel).

Contract: kernel(**full_inputs) -> full output [B, S, H] float32.

Strategy
--------
- Host: compute the (tiny) gate + group-topk routing in numpy (bit-identical
  selection to the jax reference), build per-(token,expert) combine weights,
  and dispatch: gather each expert's tokens into padded, transposed bf16
  buffers (the "all-to-all by topk_idx" of the sharding hint, done at
  input-sharding time).
- Device (SPMD over 8 cores):
    * Shared expert is TOKEN-sharded: each core runs the full SwiGLU
      (SI = 2816 = 22x128, zero padding) over its own T/8 = 512 tokens.
    * Routed experts are EXPERT-sharded with load balancing: each expert's
      token list is split into two halves; the 32 half-pieces are ranked by
      size and placed into 4 fixed-capacity slots x 8 cores so every core
      gets an equal, near-minimal amount of matmul columns.  Each (core,
      slot) holds one half-expert: its gathered tokens + that expert's
      gate/up/down weight panels.
- Host: scale per-piece outputs by routing weights, scatter-add, place the
  per-core shared slices, transpose back.

All matmuls run in bf16 with fp32 PSUM accumulation.  Weight panels are
pre-tiled on the host into the exact SBUF tile layout so each streams from
HBM once as a contiguous per-partition DMA; token tiles stay k-resident;
weights load on the sync queue, x tiles on scalar/gpsimd, outputs on
gpsimd, so no queue serializes against another's burst.  The very first
weight tiles are split into quarter-DMAs so the PE starts within ~1us.
"""

import math

import numpy as np
import ml_dtypes

H = 2048          # hidden size
I = 1408          # intermediate per routed expert
E = 16            # routed experts
G = 4             # groups
TOPK_GROUP = 2
TOP_K = 6
N_SHARED = 2
SCALE_FACTOR = 2.5
SI = I * N_SHARED   # 2816 shared intermediate
N_CORES = 8
NSLOT = 4           # routed half-expert slots per core
P = 128
KH = H // P         # 16 contraction chunks over H
MI = I // P         # 11 I chunks (routed)
MSI = SI // P       # 22 I chunks (shared)
MH = H // P         # 16 output H chunks
TS = 4096 // N_CORES  # 512 tokens per core for the shared expert
BF16 = ml_dtypes.bfloat16

_COMPILED = {}  # caps tuple -> nc
_LAST = {}      # debug/profiling handle for test.py


def _gate_host(hs, gate_weight, bias):
    """numpy replica of reference._gate (verified bit-identical selection)."""
    T = hs.shape[0]
    logits = hs @ gate_weight.T                       # [T, E] fp32
    scores = 1.0 / (1.0 + np.exp(-logits))
    sfc = scores + bias[None, :]
    gs = sfc.reshape(T, G, E // G)
    gsort = np.sort(gs, axis=-1)
    group_scores = gsort[..., -1] + gsort[..., -2]
    group_idx = np.argsort(-group_scores, axis=-1, kind="stable")[:, :TOPK_GROUP]
    gmask = np.zeros((T, G), bool)
    gmask[np.arange(T)[:, None], group_idx] = True
    smask = np.repeat(gmask, E // G, axis=1)
    tmp = np.where(smask, sfc, 0.0)
    topk_idx = np.argsort(-tmp, axis=-1, kind="stable")[:, :TOP_K]
    topk_w = np.take_along_axis(scores, topk_idx, axis=1)
    topk_w = topk_w / (topk_w.sum(-1, keepdims=True) + 1e-20) * SCALE_FACTOR
    return topk_idx.astype(np.int32), topk_w.astype(np.float32)


def _build(caps):
    """Build + compile the SPMD Bass program.

    caps : per routed slot capacity (tokens, each = 2*w with w % 4 == 0)
    """
    import concourse.mybir as mybir
    import concourse.tile as tile
    from concourse import bacc

    bf = mybir.dt.bfloat16
    f32 = mybir.dt.float32
    AF = mybir.ActivationFunctionType

    C_tot = sum(caps)
    slot_base = [sum(caps[:s]) for s in range(len(caps))]
    ws = []
    for cap in caps:
        assert cap % 8 == 0
        ws.append(cap // 2)

    nc = bacc.Bacc("TRN2", target_bir_lowering=False, debug=False,
                   num_devices=N_CORES)
    xs = nc.dram_tensor("xs", [H, TS], bf, kind="ExternalInput")
    xg = nc.dram_tensor("xg", [H, C_tot], bf, kind="ExternalInput")
    # weight panels are pre-tiled on the host to the exact SBUF tile layout
    # [tile_idx, partition, ko*128+c] so every load is a contiguous
    # per-partition stream
    wg = nc.dram_tensor("wg", [NSLOT * MI, P, KH * P], bf,
                        kind="ExternalInput")
    wu = nc.dram_tensor("wu", [NSLOT * MI, P, KH * P], bf,
                        kind="ExternalInput")
    wd = nc.dram_tensor("wd", [NSLOT * MH, P, MI * P], bf,
                        kind="ExternalInput")
    sg = nc.dram_tensor("sg", [MSI, P, KH * P], bf, kind="ExternalInput")
    su = nc.dram_tensor("su", [MSI, P, KH * P], bf, kind="ExternalInput")
    sd = nc.dram_tensor("sd", [MH, P, MSI * P], bf, kind="ExternalInput")
    ye = nc.dram_tensor("ye", [H, C_tot], bf, kind="ExternalOutput")
    ys = nc.dram_tensor("ys", [H, TS], bf, kind="ExternalOutput")

    with tile.TileContext(nc) as tc:
        with (
            tc.tile_pool(name="xb", bufs=2) as xbp,    # [128,16,cap] x blocks
            tc.tile_pool(name="x0", bufs=4) as x0p,    # [128,4,cap] slot0 x
            tc.tile_pool(name="wp", bufs=8) as wp,     # [128,16,128] g/u cols
            tc.tile_pool(name="wq", bufs=8) as wqp,    # [128,4,128] m0 quarters
            tc.tile_pool(name="wdp", bufs=4) as wdp,   # [128,11,128] down
            tc.tile_pool(name="sdp", bufs=3) as sdp,   # [128,22,128] sh down
            tc.tile_pool(name="itp", bufs=27) as itp,  # [128,512] bf16 inter
            tc.tile_pool(name="tmp", bufs=4) as tmp,   # silu temp
            tc.tile_pool(name="otp", bufs=6) as otp,   # [128,<=1024] bf16 out
            tc.tile_pool(name="pg", bufs=2, space="PSUM") as pgp,
            tc.tile_pool(name="pu", bufs=2, space="PSUM") as pup,
            tc.tile_pool(name="py", bufs=4, space="PSUM") as pyp,
        ):
            # ---------------- section order: routed slots 0..3, shared last.
            # Routed gate/up chains consume weights at ~94 GB/s (each tile
            # feeds 2*w columns) vs ~147 GB/s for shared, so starting with
            # a routed slot lets the DMA pipeline fill without stalling the
            # PE.  Each section's x tiles are drip-fed from inside the
            # previous section's down loop (one chunk per M iteration) so
            # the in-order scalar queue interleaves outputs and prefetch.
            def load_gu(pool, tagname, dram, row, nm, split):
                if split:
                    parts = []
                    for q in range(4):
                        t = wqp.tile([P, 4, P], bf, name=f"{nm}_{q}",
                                     tag="wq")
                        nc.sync.dma_start(
                            t[:],
                            dram[row].rearrange(
                                "p (ko c) -> p ko c", c=P)[:, 4 * q:4 * q + 4, :])
                        parts.append(t)
                    return parts
                t = pool.tile([P, KH, P], bf, name=nm, tag=tagname)
                nc.sync.dma_start(
                    t[:], dram[row].rearrange("p (ko c) -> p ko c", c=P))
                return t

            def gu_slice(t, k):
                if isinstance(t, list):
                    return t[k // 4][:, k % 4, :]
                return t[:, k, :]

            x_blocks = {}  # s -> big tile [P, 16, cap] ("sh" for shared)

            def emit_xg_block(s):
                # one strided mega-DMA loads all 16 contraction chunks of
                # this slot's gathered tokens (quarters the ~600ns/issue
                # cost of fine-grained loads)
                cap = caps[s]
                t = xbp.tile([P, KH, cap], bf, name=f"xgb{s}", tag="xb")
                nc.scalar.dma_start(
                    t[:], xg.ap().rearrange("(k p) c -> p k c", p=P)[
                        :, :, slot_base[s]:slot_base[s] + cap])
                x_blocks[s] = t

            def emit_xs_block():
                t = xbp.tile([P, KH, TS], bf, name="xsb", tag="xb")
                nc.gpsimd.dma_start(
                    t[:], xs.ap().rearrange("(k p) c -> p k c", p=P))
                x_blocks["sh"] = t

            def emit_routed(s, first, prefetch):
                w = ws[s]
                cap = caps[s]
                b0 = slot_base[s]
                inter = {}
                for m in range(MI):
                    if m == 5:
                        # prefetch the next section's x block while this
                        # slot still has ~60us of gate/up left
                        if prefetch == "sh":
                            emit_xs_block()
                        else:
                            emit_xg_block(prefetch)
                    split = first and m == 0
                    if split:
                        wgt, wut = first  # preloaded interleaved at top
                    else:
                        wgt = load_gu(wp, "wp", wg, s * MI + m,
                                      f"wgt{s}_{m}", False)
                        wut = load_gu(wp, "wp", wu, s * MI + m,
                                      f"wut{s}_{m}", False)
                    if split:
                        # k-outer warmup: each arriving x chunk feeds 4
                        # matmuls (gate/up x j0/j1) so the PE keeps pace
                        # with the DMA feed during the first pass.
                        pst = {}
                        for gu in range(2):
                            pool_, wt = [(pgp, "pg"), (pup, "pu")][gu]
                            for j in range(2):
                                pst[(gu, j)] = pool_.tile(
                                    [P, TS], f32, name=f"pw{s}{gu}{j}",
                                    tag=wt)
                        for k in range(KH):
                            for gu, wtile in ((0, wgt), (1, wut)):
                                for j in range(2):
                                    nc.tensor.matmul(
                                        pst[(gu, j)][:, :w],
                                        gu_slice(wtile, k),
                                        x_slice(s, k)[:, j * w:(j + 1) * w],
                                        start=(k == 0), stop=(k == KH - 1))
                        for j in range(2):
                            st = tmp.tile([P, TS], bf, name=f"st{s}_0{j}",
                                          tag="tmp")
                            nc.scalar.activation(st[:, :w],
                                                 pst[(0, j)][:, :w], AF.Silu)
                            it = itp.tile([P, TS], bf, name=f"it{s}_0{j}",
                                          tag="it")
                            nc.vector.tensor_mul(it[:, :w], st[:, :w],
                                                 pst[(1, j)][:, :w])
                            inter[(0, j)] = it
                        continue
                    for j in range(2):
                        psg = pgp.tile([P, TS], f32, name=f"psg{s}_{m}{j}",
                                       tag="pg")
                        for k in range(KH):
                            nc.tensor.matmul(
                                psg[:, :w], gu_slice(wgt, k),
                                x_slice(s, k)[:, j * w:(j + 1) * w],
                                start=(k == 0), stop=(k == KH - 1))
                        st = tmp.tile([P, TS], bf, name=f"st{s}_{m}{j}",
                                      tag="tmp")
                        nc.scalar.activation(st[:, :w], psg[:, :w], AF.Silu)
                        psu = pup.tile([P, TS], f32, name=f"psu{s}_{m}{j}",
                                       tag="pu")
                        for k in range(KH):
                            nc.tensor.matmul(
                                psu[:, :w], gu_slice(wut, k),
                                x_slice(s, k)[:, j * w:(j + 1) * w],
                                start=(k == 0), stop=(k == KH - 1))
                        it = itp.tile([P, TS], bf, name=f"it{s}_{m}{j}",
                                      tag="it")
                        nc.vector.tensor_mul(it[:, :w], st[:, :w],
                                             psu[:, :w])
                        inter[(m, j)] = it
                for M in range(MH):
                    wdt = wdp.tile([P, MI, P], bf, name=f"wdt{s}_{M}",
                                   tag="wdt")
                    nc.sync.dma_start(
                        wdt[:], wd[s * MH + M].rearrange(
                            "p (ko c) -> p ko c", c=P))
                    ot = otp.tile([P, 2 * w], bf, name=f"ot{s}_{M}", tag="ot")
                    for j in range(2):
                        psy = pyp.tile([P, TS], f32, name=f"psy{s}_{M}{j}",
                                       tag="py")
                        for K in range(MI):
                            nc.tensor.matmul(
                                psy[:, :w], wdt[:, K, :],
                                inter[(K, j)][:, :w],
                                start=(K == 0), stop=(K == MI - 1))
                        nc.vector.tensor_copy(
                            ot[:, j * w:(j + 1) * w], psy[:, :w])
                    nc.scalar.dma_start(
                        ye[M * P:(M + 1) * P, b0:b0 + cap], ot[:])

            def emit_shared():
                sint = []
                for m in range(MSI):
                    sgt = load_gu(wp, "wp", sg, m, f"sgt{m}", False)
                    sut = load_gu(wp, "wp", su, m, f"sut{m}", False)
                    psg = pgp.tile([P, TS], f32, name=f"psgs{m}", tag="pg")
                    for k in range(KH):
                        nc.tensor.matmul(psg[:], sgt[:, k, :],
                                         x_blocks["sh"][:, k, :],
                                         start=(k == 0), stop=(k == KH - 1))
                    st = tmp.tile([P, TS], bf, name=f"sts{m}", tag="tmp")
                    nc.scalar.activation(st[:], psg[:], AF.Silu)
                    psu = pup.tile([P, TS], f32, name=f"psus{m}", tag="pu")
                    for k in range(KH):
                        nc.tensor.matmul(psu[:], sut[:, k, :],
                                         x_blocks["sh"][:, k, :],
                                         start=(k == 0), stop=(k == KH - 1))
                    it = itp.tile([P, TS], bf, name=f"si{m}", tag="it")
                    nc.vector.tensor_mul(it[:], st[:], psu[:])
                    sint.append(it)
                for M in range(MH):
                    sdt = sdp.tile([P, MSI, P], bf, name=f"sdt{M}", tag="sdt")
                    nc.sync.dma_start(
                        sdt[:], sd[M].rearrange("p (ko c) -> p ko c", c=P))
                    psy = pyp.tile([P, TS], f32, name=f"psys{M}", tag="py")
                    for K in range(MSI):
                        nc.tensor.matmul(psy[:], sdt[:, K, :], sint[K][:],
                                         start=(K == 0), stop=(K == MSI - 1))
                    ot = otp.tile([P, TS], bf, name=f"ots{M}", tag="ot")
                    nc.vector.tensor_copy(ot[:], psy[:])
                    nc.scalar.dma_start(ys[M * P:(M + 1) * P, :], ot[:])

            # slot0 startup: m0 weight quarters on sync, x in 4 block-DMAs
            # on scalar, interleaved so the k-outer warmup streams at the
            # DMA feed rate from the first landing block.
            wgt0, wut0 = [], []
            x0_blocks = []
            for q in range(4):
                for dram, lst, nm in ((wg, wgt0, "wgt0"), (wu, wut0, "wut0")):
                    t = wqp.tile([P, 4, P], bf, name=f"{nm}_{q}", tag="wq")
                    nc.sync.dma_start(
                        t[:],
                        dram[0].rearrange(
                            "p (ko c) -> p ko c", c=P)[:, 4 * q:4 * q + 4, :])
                    lst.append(t)
                t = x0p.tile([P, 4, caps[0]], bf, name=f"x0b{q}", tag="x0")
                nc.scalar.dma_start(
                    t[:], xg.ap().rearrange("(k p) c -> p k c", p=P)[
                        :, 4 * q:4 * q + 4, 0:caps[0]])
                x0_blocks.append(t)

            def x_slice(s, k):
                if s == 0:
                    return x0_blocks[k // 4][:, k % 4, :]
                return x_blocks[s][:, k, :]

            emit_routed(0, (wgt0, wut0), 1)
            emit_routed(1, False, 2)
            emit_routed(2, False, 3)
            emit_routed(3, False, "sh")
            emit_shared()

    nc.compile()
    return nc


def _get_compiled(caps):
    key = tuple(caps)
    if key not in _COMPILED:
        _COMPILED[key] = _build(caps)
    return _COMPILED[key]


def _tile_gu(wmat, mi):  # [mi*P, H] -> [mi, P, KH*P] : (m, c_i, ko_h*P + p_i)
    return np.ascontiguousarray(
        wmat.reshape(mi, P, KH, P).transpose(0, 3, 2, 1)
    ).reshape(mi, P, KH * P).astype(BF16)


def _tile_dn(wmat, mi):  # [H, mi*P] -> [MH, P, mi*P]
    return np.ascontiguousarray(
        wmat.reshape(MH, P, mi, P).transpose(0, 3, 2, 1)
    ).reshape(MH, P, mi * P).astype(BF16)


def kernel(hidden_states, gate_weight, e_score_correction_bias,
           gate_proj, up_proj, down_proj,
           shared_gate_w, shared_up_w, shared_down_w):
    from concourse.bass_utils import run_bass_kernel_spmd

    hs = np.asarray(hidden_states, dtype=np.float32)
    B, S, Hh = hs.shape
    assert Hh == H
    hsf = np.ascontiguousarray(hs.reshape(-1, H))
    T = hsf.shape[0]
    assert T == N_CORES * TS
    gate_weight = np.asarray(gate_weight, np.float32)
    bias = np.asarray(e_score_correction_bias, np.float32)
    gate_proj = np.asarray(gate_proj, np.float32)
    up_proj = np.asarray(up_proj, np.float32)
    down_proj = np.asarray(down_proj, np.float32)
    shared_gate_w = np.asarray(shared_gate_w, np.float32)
    shared_up_w = np.asarray(shared_up_w, np.float32)
    shared_down_w = np.asarray(shared_down_w, np.float32)

    # ---- routing on host ----
    topk_idx, topk_w = _gate_host(hsf, gate_weight, bias)
    comb = np.zeros((T, E), np.float32)
    np.add.at(comb, (np.arange(T)[:, None], topk_idx), topk_w)
    sel = np.zeros((T, E), bool)
    sel[np.arange(T)[:, None], topk_idx] = True
    idx_e = [np.nonzero(sel[:, e])[0] for e in range(E)]

    # split each expert's tokens in half; rank the 32 pieces by size and
    # deal them into NSLOT slots of 8 cores so per-slot capacity (uniform
    # across cores under SPMD) hugs the max piece of its rank
    pieces = []
    for e in range(E):
        n = len(idx_e[e])
        h1 = (n + 1) // 2
        pieces.append((e, idx_e[e][:h1]))
        pieces.append((e, idx_e[e][h1:]))
    pieces.sort(key=lambda p: -len(p[1]))
    assert len(pieces) == NSLOT * N_CORES
    caps, assign = [], []
    for r in range(NSLOT):
        grp = pieces[N_CORES * r:N_CORES * (r + 1)]
        cap = max(8, -(-max(len(p[1]) for p in grp) // 8) * 8)
        caps.append(cap)
        assign.append(grp)
    slot_base = [sum(caps[:s]) for s in range(NSLOT)]
    C_tot = sum(caps)

    # ---- host-side dispatch (shard + transpose + bf16 cast) ----
    xsT = np.ascontiguousarray(hsf.T).astype(BF16)          # [H, T]

    gu_g = {e: _tile_gu(gate_proj[e], MI) for e in range(E)}
    gu_u = {e: _tile_gu(up_proj[e], MI) for e in range(E)}
    dn = {e: _tile_dn(down_proj[e], MI) for e in range(E)}
    sg_p = _tile_gu(shared_gate_w, MSI)
    su_p = _tile_gu(shared_up_w, MSI)
    sd_p = _tile_dn(shared_down_w, MSI)

    in_maps = []
    for c in range(N_CORES):
        xg_c = np.zeros((H, C_tot), BF16)
        for r in range(NSLOT):
            e, idx = assign[r][c]
            xg_c[:, slot_base[r]:slot_base[r] + len(idx)] = xsT[:, idx]
        wg_c = np.concatenate([gu_g[assign[r][c][0]] for r in range(NSLOT)])
        wu_c = np.concatenate([gu_u[assign[r][c][0]] for r in range(NSLOT)])
        wd_c = np.concatenate([dn[assign[r][c][0]] for r in range(NSLOT)])
        xs_c = np.ascontiguousarray(xsT[:, c * TS:(c + 1) * TS])
        in_maps.append({
            "xs": xs_c, "xg": xg_c,
            "wg": wg_c, "wu": wu_c, "wd": wd_c,
            "sg": sg_p, "su": su_p, "sd": sd_p,
        })

    nc = _get_compiled(caps)
    results = run_bass_kernel_spmd(nc, in_maps, core_ids=list(range(N_CORES)))

    _LAST.clear()
    _LAST.update(nc=nc, in_maps=in_maps, results=results, caps=caps,
                 assign=assign)

    # ---- host-side combine ----
    outT = np.empty((H, T), np.float32)
    for c in range(N_CORES):
        outT[:, c * TS:(c + 1) * TS] = results.results[c]["ys"]
    for c in range(N_CORES):
        ye = results.results[c]["ye"].astype(np.float32)
        for r in range(NSLOT):
            e, idx = assign[r][c]
            if len(idx) == 0:
                continue
            b0 = slot_base[r]
            we = comb[idx, e]
            outT[:, idx] += ye[:, b0:b0 + len(idx)] * we[None, :]

    return np.ascontiguousarray(outT.T).reshape(B, S, H).astype(np.float32)


# revision 21
# speedup vs baseline: 1.0134x; 1.0005x over previous
"""MoE routing kernel for Trainium2 (8 NeuronCores, SPMD expert-parallel).

Contract: kernel(**full_inputs) -> full output [B, S, H] float32.

Strategy
--------
- Host: compute the (tiny) gate + group-topk routing in numpy (bit-identical
  selection to the jax reference), build per-(token,expert) combine weights,
  and dispatch: gather each expert's tokens into padded, transposed bf16
  buffers (the "all-to-all by topk_idx" of the sharding hint, done at
  input-sharding time).
- Device (SPMD over 8 cores):
    * Shared expert is TOKEN-sharded: each core runs the full SwiGLU
      (SI = 2816 = 22x128, zero padding) over its own T/8 = 512 tokens.
    * Routed experts are EXPERT-sharded with load balancing: each expert's
      token list is split into two halves; the 32 half-pieces are ranked by
      size and placed into 4 fixed-capacity slots x 8 cores so every core
      gets an equal, near-minimal amount of matmul columns.  Each (core,
      slot) holds one half-expert: its gathered tokens + that expert's
      gate/up/down weight panels.
- Host: scale per-piece outputs by routing weights, scatter-add, place the
  per-core shared slices, transpose back.

All matmuls run in bf16 with fp32 PSUM accumulation.  Weight panels are
pre-tiled on the host into the exact SBUF tile layout so each streams from
HBM once as a contiguous per-partition DMA; token tiles stay k-resident;
weights load on the sync queue, x tiles on scalar/gpsimd, outputs on
gpsimd, so no queue serializes against another's burst.  The very first
weight tiles are split into quarter-DMAs so the PE starts within ~1us.
"""

import math

import numpy as np
import ml_dtypes

H = 2048          # hidden size
I = 1408          # intermediate per routed expert
E = 16            # routed experts
G = 4             # groups
TOPK_GROUP = 2
TOP_K = 6
N_SHARED = 2
SCALE_FACTOR = 2.5
SI = I * N_SHARED   # 2816 shared intermediate
N_CORES = 8
NSLOT = 4           # routed half-expert slots per core
P = 128
KH = H // P         # 16 contraction chunks over H
MI = I // P         # 11 I chunks (routed)
MSI = SI // P       # 22 I chunks (shared)
MH = H // P         # 16 output H chunks
TS = 4096 // N_CORES  # 512 tokens per core for the shared expert
BF16 = ml_dtypes.bfloat16

_COMPILED = {}  # caps tuple -> nc
_LAST = {}      # debug/profiling handle for test.py


def _gate_host(hs, gate_weight, bias):
    """numpy replica of reference._gate (verified bit-identical selection)."""
    T = hs.shape[0]
    logits = hs @ gate_weight.T                       # [T, E] fp32
    scores = 1.0 / (1.0 + np.exp(-logits))
    sfc = scores + bias[None, :]
    gs = sfc.reshape(T, G, E // G)
    gsort = np.sort(gs, axis=-1)
    group_scores = gsort[..., -1] + gsort[..., -2]
    group_idx = np.argsort(-group_scores, axis=-1, kind="stable")[:, :TOPK_GROUP]
    gmask = np.zeros((T, G), bool)
    gmask[np.arange(T)[:, None], group_idx] = True
    smask = np.repeat(gmask, E // G, axis=1)
    tmp = np.where(smask, sfc, 0.0)
    topk_idx = np.argsort(-tmp, axis=-1, kind="stable")[:, :TOP_K]
    topk_w = np.take_along_axis(scores, topk_idx, axis=1)
    topk_w = topk_w / (topk_w.sum(-1, keepdims=True) + 1e-20) * SCALE_FACTOR
    return topk_idx.astype(np.int32), topk_w.astype(np.float32)


def _build(caps):
    """Build + compile the SPMD Bass program.

    caps : per routed slot capacity (tokens, each = 2*w with w % 4 == 0)
    """
    import concourse.mybir as mybir
    import concourse.tile as tile
    from concourse import bacc

    bf = mybir.dt.bfloat16
    f32 = mybir.dt.float32
    AF = mybir.ActivationFunctionType

    C_tot = sum(caps)
    slot_base = [sum(caps[:s]) for s in range(len(caps))]
    ws = []
    for cap in caps:
        assert cap % 8 == 0
        ws.append(cap // 2)

    nc = bacc.Bacc("TRN2", target_bir_lowering=False, debug=False,
                   num_devices=N_CORES)
    # x is pre-arranged on the host into the SBUF tile layout
    # [128 partitions, KH * cap] (k-major, token minor) so every x load is
    # one fully-contiguous per-partition stream at full HBM rate
    xs = nc.dram_tensor("xs", [P, KH * TS], bf, kind="ExternalInput")
    xg = nc.dram_tensor("xg", [P, KH * C_tot], bf, kind="ExternalInput")
    # weight panels are pre-tiled on the host to the exact SBUF tile layout
    # [tile_idx, partition, ko*128+c] so every load is a contiguous
    # per-partition stream
    wg = nc.dram_tensor("wg", [NSLOT * MI, P, KH * P], bf,
                        kind="ExternalInput")
    wu = nc.dram_tensor("wu", [NSLOT * MI, P, KH * P], bf,
                        kind="ExternalInput")
    wd = nc.dram_tensor("wd", [NSLOT * MH, P, MI * P], bf,
                        kind="ExternalInput")
    sg = nc.dram_tensor("sg", [MSI, P, KH * P], bf, kind="ExternalInput")
    su = nc.dram_tensor("su", [MSI, P, KH * P], bf, kind="ExternalInput")
    sd = nc.dram_tensor("sd", [MH, P, MSI * P], bf, kind="ExternalInput")
    ye = nc.dram_tensor("ye", [H, C_tot], bf, kind="ExternalOutput")
    ys = nc.dram_tensor("ys", [H, TS], bf, kind="ExternalOutput")

    with tile.TileContext(nc) as tc:
        with (
            tc.tile_pool(name="xb", bufs=2) as xbp,    # [128,16,cap] x blocks
            tc.tile_pool(name="x0", bufs=4) as x0p,    # [128,4,cap] slot0 x
            tc.tile_pool(name="wp", bufs=8) as wp,     # [128,16,128] g/u cols
            tc.tile_pool(name="wq", bufs=8) as wqp,    # [128,4,128] m0 quarters
            tc.tile_pool(name="wdp", bufs=4) as wdp,   # [128,11,128] down
            tc.tile_pool(name="sdp", bufs=3) as sdp,   # [128,22,128] sh down
            tc.tile_pool(name="itp", bufs=27) as itp,  # [128,512] bf16 inter
            tc.tile_pool(name="tmp", bufs=4) as tmp,   # silu temp
            tc.tile_pool(name="otp", bufs=6) as otp,   # [128,<=1024] bf16 out
            tc.tile_pool(name="pg", bufs=2, space="PSUM") as pgp,
            tc.tile_pool(name="pu", bufs=2, space="PSUM") as pup,
            tc.tile_pool(name="py", bufs=4, space="PSUM") as pyp,
        ):
            # ---------------- section order: routed slots 0..3, shared last.
            # Routed gate/up chains consume weights at ~94 GB/s (each tile
            # feeds 2*w columns) vs ~147 GB/s for shared, so starting with
            # a routed slot lets the DMA pipeline fill without stalling the
            # PE.  Each section's x tiles are drip-fed from inside the
            # previous section's down loop (one chunk per M iteration) so
            # the in-order scalar queue interleaves outputs and prefetch.
            def load_gu(pool, tagname, dram, row, nm, split):
                if split:
                    parts = []
                    for q in range(4):
                        t = wqp.tile([P, 4, P], bf, name=f"{nm}_{q}",
                                     tag="wq")
                        nc.sync.dma_start(
                            t[:],
                            dram[row].rearrange(
                                "p (ko c) -> p ko c", c=P)[:, 4 * q:4 * q + 4, :])
                        parts.append(t)
                    return parts
                t = pool.tile([P, KH, P], bf, name=nm, tag=tagname)
                nc.sync.dma_start(
                    t[:], dram[row].rearrange("p (ko c) -> p ko c", c=P))
                return t

            def gu_slice(t, k):
                if isinstance(t, list):
                    return t[k // 4][:, k % 4, :]
                return t[:, k, :]

            x_blocks = {}  # s -> big tile [P, 16, cap] ("sh" for shared)

            def emit_xg_block(s):
                # one contiguous mega-DMA loads all 16 contraction chunks
                # of this slot's gathered tokens
                cap = caps[s]
                off = KH * slot_base[s]
                t = xbp.tile([P, KH, cap], bf, name=f"xgb{s}", tag="xb")
                nc.scalar.dma_start(
                    t[:], xg[:, off:off + KH * cap].rearrange(
                        "p (k c) -> p k c", c=cap))
                x_blocks[s] = t

            def emit_xs_block():
                t = xbp.tile([P, KH, TS], bf, name="xsb", tag="xb")
                nc.gpsimd.dma_start(
                    t[:], xs.ap().rearrange("p (k c) -> p k c", c=TS))
                x_blocks["sh"] = t

            def emit_routed(s, first, prefetch):
                w = ws[s]
                cap = caps[s]
                b0 = slot_base[s]
                inter = {}
                for m in range(MI):
                    if m == 5:
                        # prefetch the next section's x block while this
                        # slot still has ~60us of gate/up left
                        if prefetch == "sh":
                            emit_xs_block()
                        else:
                            emit_xg_block(prefetch)
                    split = first and m == 0
                    if split:
                        wgt, wut = first  # preloaded interleaved at top
                    else:
                        wgt = load_gu(wp, "wp", wg, s * MI + m,
                                      f"wgt{s}_{m}", False)
                        wut = load_gu(wp, "wp", wu, s * MI + m,
                                      f"wut{s}_{m}", False)
                    if split:
                        # k-outer warmup: each arriving x chunk feeds 4
                        # matmuls (gate/up x j0/j1) so the PE keeps pace
                        # with the DMA feed during the first pass.
                        pst = {}
                        for gu in range(2):
                            pool_, wt = [(pgp, "pg"), (pup, "pu")][gu]
                            for j in range(2):
                                pst[(gu, j)] = pool_.tile(
                                    [P, TS], f32, name=f"pw{s}{gu}{j}",
                                    tag=wt)
                        for k in range(KH):
                            for gu, wtile in ((0, wgt), (1, wut)):
                                for j in range(2):
                                    nc.tensor.matmul(
                                        pst[(gu, j)][:, :w],
                                        gu_slice(wtile, k),
                                        x_slice(s, k)[:, j * w:(j + 1) * w],
                                        start=(k == 0), stop=(k == KH - 1))
                        for j in range(2):
                            st = tmp.tile([P, TS], bf, name=f"st{s}_0{j}",
                                          tag="tmp")
                            nc.scalar.activation(st[:, :w],
                                                 pst[(0, j)][:, :w], AF.Silu)
                            it = itp.tile([P, TS], bf, name=f"it{s}_0{j}",
                                          tag="it")
                            nc.vector.tensor_mul(it[:, :w], st[:, :w],
                                                 pst[(1, j)][:, :w])
                            inter[(0, j)] = it
                        continue
                    for j in range(2):
                        psg = pgp.tile([P, TS], f32, name=f"psg{s}_{m}{j}",
                                       tag="pg")
                        for k in range(KH):
                            nc.tensor.matmul(
                                psg[:, :w], gu_slice(wgt, k),
                                x_slice(s, k)[:, j * w:(j + 1) * w],
                                start=(k == 0), stop=(k == KH - 1))
                        st = tmp.tile([P, TS], bf, name=f"st{s}_{m}{j}",
                                      tag="tmp")
                        nc.scalar.activation(st[:, :w], psg[:, :w], AF.Silu)
                        psu = pup.tile([P, TS], f32, name=f"psu{s}_{m}{j}",
                                       tag="pu")
                        for k in range(KH):
                            nc.tensor.matmul(
                                psu[:, :w], gu_slice(wut, k),
                                x_slice(s, k)[:, j * w:(j + 1) * w],
                                start=(k == 0), stop=(k == KH - 1))
                        it = itp.tile([P, TS], bf, name=f"it{s}_{m}{j}",
                                      tag="it")
                        nc.vector.tensor_mul(it[:, :w], st[:, :w],
                                             psu[:, :w])
                        inter[(m, j)] = it
                for M in range(MH):
                    wdt = wdp.tile([P, MI, P], bf, name=f"wdt{s}_{M}",
                                   tag="wdt")
                    nc.sync.dma_start(
                        wdt[:], wd[s * MH + M].rearrange(
                            "p (ko c) -> p ko c", c=P))
                    ot = otp.tile([P, 2 * w], bf, name=f"ot{s}_{M}", tag="ot")
                    for j in range(2):
                        psy = pyp.tile([P, TS], f32, name=f"psy{s}_{M}{j}",
                                       tag="py")
                        for K in range(MI):
                            nc.tensor.matmul(
                                psy[:, :w], wdt[:, K, :],
                                inter[(K, j)][:, :w],
                                start=(K == 0), stop=(K == MI - 1))
                        nc.vector.tensor_copy(
                            ot[:, j * w:(j + 1) * w], psy[:, :w])
                    nc.scalar.dma_start(
                        ye[M * P:(M + 1) * P, b0:b0 + cap], ot[:])

            def emit_shared():
                sint = []
                for m in range(MSI):
                    sgt = load_gu(wp, "wp", sg, m, f"sgt{m}", False)
                    sut = load_gu(wp, "wp", su, m, f"sut{m}", False)
                    psg = pgp.tile([P, TS], f32, name=f"psgs{m}", tag="pg")
                    for k in range(KH):
                        nc.tensor.matmul(psg[:], sgt[:, k, :],
                                         x_blocks["sh"][:, k, :],
                                         start=(k == 0), stop=(k == KH - 1))
                    st = tmp.tile([P, TS], bf, name=f"sts{m}", tag="tmp")
                    nc.scalar.activation(st[:], psg[:], AF.Silu)
                    psu = pup.tile([P, TS], f32, name=f"psus{m}", tag="pu")
                    for k in range(KH):
                        nc.tensor.matmul(psu[:], sut[:, k, :],
                                         x_blocks["sh"][:, k, :],
                                         start=(k == 0), stop=(k == KH - 1))
                    it = itp.tile([P, TS], bf, name=f"si{m}", tag="it")
                    nc.vector.tensor_mul(it[:], st[:], psu[:])
                    sint.append(it)
                for M in range(MH):
                    sdt = sdp.tile([P, MSI, P], bf, name=f"sdt{M}", tag="sdt")
                    nc.sync.dma_start(
                        sdt[:], sd[M].rearrange("p (ko c) -> p ko c", c=P))
                    psy = pyp.tile([P, TS], f32, name=f"psys{M}", tag="py")
                    for K in range(MSI):
                        nc.tensor.matmul(psy[:], sdt[:, K, :], sint[K][:],
                                         start=(K == 0), stop=(K == MSI - 1))
                    ot = otp.tile([P, TS], bf, name=f"ots{M}", tag="ot")
                    nc.vector.tensor_copy(ot[:], psy[:])
                    nc.scalar.dma_start(ys[M * P:(M + 1) * P, :], ot[:])

            # slot0 startup: m0 weight quarters on sync, x in 4 block-DMAs
            # on scalar, interleaved so the k-outer warmup streams at the
            # DMA feed rate from the first landing block.
            wgt0, wut0 = [], []
            x0_blocks = []
            for q in range(4):
                for dram, lst, nm in ((wg, wgt0, "wgt0"), (wu, wut0, "wut0")):
                    t = wqp.tile([P, 4, P], bf, name=f"{nm}_{q}", tag="wq")
                    nc.sync.dma_start(
                        t[:],
                        dram[0].rearrange(
                            "p (ko c) -> p ko c", c=P)[:, 4 * q:4 * q + 4, :])
                    lst.append(t)
                t = x0p.tile([P, 4, caps[0]], bf, name=f"x0b{q}", tag="x0")
                nc.scalar.dma_start(
                    t[:], xg[:, 4 * q * caps[0]:(4 * q + 4) * caps[0]]
                    .rearrange("p (k c) -> p k c", c=caps[0]))
                x0_blocks.append(t)

            def x_slice(s, k):
                if s == 0:
                    return x0_blocks[k // 4][:, k % 4, :]
                return x_blocks[s][:, k, :]

            emit_routed(0, (wgt0, wut0), 1)
            emit_routed(1, False, 2)
            emit_routed(2, False, 3)
            emit_routed(3, False, "sh")
            emit_shared()

    nc.compile()
    return nc


def _get_compiled(caps):
    key = tuple(caps)
    if key not in _COMPILED:
        _COMPILED[key] = _build(caps)
    return _COMPILED[key]


def _tile_gu(wmat, mi):  # [mi*P, H] -> [mi, P, KH*P] : (m, c_i, ko_h*P + p_i)
    return np.ascontiguousarray(
        wmat.reshape(mi, P, KH, P).transpose(0, 3, 2, 1)
    ).reshape(mi, P, KH * P).astype(BF16)


def _tile_dn(wmat, mi):  # [H, mi*P] -> [MH, P, mi*P]
    return np.ascontiguousarray(
        wmat.reshape(MH, P, mi, P).transpose(0, 3, 2, 1)
    ).reshape(MH, P, mi * P).astype(BF16)


def kernel(hidden_states, gate_weight, e_score_correction_bias,
           gate_proj, up_proj, down_proj,
           shared_gate_w, shared_up_w, shared_down_w):
    from concourse.bass_utils import run_bass_kernel_spmd

    hs = np.asarray(hidden_states, dtype=np.float32)
    B, S, Hh = hs.shape
    assert Hh == H
    hsf = np.ascontiguousarray(hs.reshape(-1, H))
    T = hsf.shape[0]
    assert T == N_CORES * TS
    gate_weight = np.asarray(gate_weight, np.float32)
    bias = np.asarray(e_score_correction_bias, np.float32)
    gate_proj = np.asarray(gate_proj, np.float32)
    up_proj = np.asarray(up_proj, np.float32)
    down_proj = np.asarray(down_proj, np.float32)
    shared_gate_w = np.asarray(shared_gate_w, np.float32)
    shared_up_w = np.asarray(shared_up_w, np.float32)
    shared_down_w = np.asarray(shared_down_w, np.float32)

    # ---- routing on host ----
    topk_idx, topk_w = _gate_host(hsf, gate_weight, bias)
    comb = np.zeros((T, E), np.float32)
    np.add.at(comb, (np.arange(T)[:, None], topk_idx), topk_w)
    sel = np.zeros((T, E), bool)
    sel[np.arange(T)[:, None], topk_idx] = True
    idx_e = [np.nonzero(sel[:, e])[0] for e in range(E)]

    # split each expert's tokens in half; rank the 32 pieces by size and
    # deal them into NSLOT slots of 8 cores so per-slot capacity (uniform
    # across cores under SPMD) hugs the max piece of its rank
    pieces = []
    for e in range(E):
        n = len(idx_e[e])
        h1 = (n + 1) // 2
        pieces.append((e, idx_e[e][:h1]))
        pieces.append((e, idx_e[e][h1:]))
    pieces.sort(key=lambda p: -len(p[1]))
    assert len(pieces) == NSLOT * N_CORES
    caps, assign = [], []
    for r in range(NSLOT):
        grp = pieces[N_CORES * r:N_CORES * (r + 1)]
        cap = max(8, -(-max(len(p[1]) for p in grp) // 8) * 8)
        caps.append(cap)
        assign.append(grp)
    slot_base = [sum(caps[:s]) for s in range(NSLOT)]
    C_tot = sum(caps)

    # ---- host-side dispatch (shard + transpose + bf16 cast) ----
    xsT = np.ascontiguousarray(hsf.T).astype(BF16)          # [H, T]

    gu_g = {e: _tile_gu(gate_proj[e], MI) for e in range(E)}
    gu_u = {e: _tile_gu(up_proj[e], MI) for e in range(E)}
    dn = {e: _tile_dn(down_proj[e], MI) for e in range(E)}
    sg_p = _tile_gu(shared_gate_w, MSI)
    su_p = _tile_gu(shared_up_w, MSI)
    sd_p = _tile_dn(shared_down_w, MSI)

    def x_tilelayout(xpiece, cap):
        # [H, n] -> [128, KH*cap] (k-major, token minor), zero-padded
        n = xpiece.shape[1]
        out = np.zeros((P, KH, cap), BF16)
        out[:, :, :n] = xpiece.reshape(KH, P, n).transpose(1, 0, 2)
        return out.reshape(P, KH * cap)

    in_maps = []
    for c in range(N_CORES):
        xg_c = np.concatenate(
            [x_tilelayout(xsT[:, assign[r][c][1]], caps[r])
             for r in range(NSLOT)], axis=1)
        wg_c = np.concatenate([gu_g[assign[r][c][0]] for r in range(NSLOT)])
        wu_c = np.concatenate([gu_u[assign[r][c][0]] for r in range(NSLOT)])
        wd_c = np.concatenate([dn[assign[r][c][0]] for r in range(NSLOT)])
        xs_c = x_tilelayout(xsT[:, c * TS:(c + 1) * TS], TS)
        in_maps.append({
            "xs": xs_c, "xg": xg_c,
            "wg": wg_c, "wu": wu_c, "wd": wd_c,
            "sg": sg_p, "su": su_p, "sd": sd_p,
        })

    nc = _get_compiled(caps)
    results = run_bass_kernel_spmd(nc, in_maps, core_ids=list(range(N_CORES)))

    _LAST.clear()
    _LAST.update(nc=nc, in_maps=in_maps, results=results, caps=caps,
                 assign=assign)

    # ---- host-side combine ----
    outT = np.empty((H, T), np.float32)
    for c in range(N_CORES):
        outT[:, c * TS:(c + 1) * TS] = results.results[c]["ys"]
    for c in range(N_CORES):
        ye = results.results[c]["ye"].astype(np.float32)
        for r in range(NSLOT):
            e, idx = assign[r][c]
            if len(idx) == 0:
                continue
            b0 = slot_base[r]
            we = comb[idx, e]
            outT[:, idx] += ye[:, b0:b0 + len(idx)] * we[None, :]

    return np.ascontiguousarray(outT.T).reshape(B, S, H).astype(np.float32)


# revision 23
# speedup vs baseline: 1.0195x; 1.0060x over previous
"""MoE routing kernel for Trainium2 (8 NeuronCores, SPMD expert-parallel).

Contract: kernel(**full_inputs) -> full output [B, S, H] float32.

Strategy
--------
- Host: compute the (tiny) gate + group-topk routing in numpy (bit-identical
  selection to the jax reference), build per-(token,expert) combine weights,
  and dispatch: gather each expert's tokens into padded, transposed bf16
  buffers (the "all-to-all by topk_idx" of the sharding hint, done at
  input-sharding time).
- Device (SPMD over 8 cores):
    * Shared expert is TOKEN-sharded: each core runs the full SwiGLU
      (SI = 2816 = 22x128, zero padding) over its own T/8 = 512 tokens.
    * Routed experts are EXPERT-sharded with load balancing: each expert's
      token list is split into two halves; the 32 half-pieces are ranked by
      size and placed into 4 fixed-capacity slots x 8 cores so every core
      gets an equal, near-minimal amount of matmul columns.  Each (core,
      slot) holds one half-expert: its gathered tokens + that expert's
      gate/up/down weight panels.
- Host: scale per-piece outputs by routing weights, scatter-add, place the
  per-core shared slices, transpose back.

All matmuls run in bf16 with fp32 PSUM accumulation.  Weight panels are
pre-tiled on the host into the exact SBUF tile layout so each streams from
HBM once as a contiguous per-partition DMA; token tiles stay k-resident;
weights load on the sync queue, x tiles on scalar/gpsimd, outputs on
gpsimd, so no queue serializes against another's burst.  The very first
weight tiles are split into quarter-DMAs so the PE starts within ~1us.
"""

import math

import numpy as np
import ml_dtypes

H = 2048          # hidden size
I = 1408          # intermediate per routed expert
E = 16            # routed experts
G = 4             # groups
TOPK_GROUP = 2
TOP_K = 6
N_SHARED = 2
SCALE_FACTOR = 2.5
SI = I * N_SHARED   # 2816 shared intermediate
N_CORES = 8
NSLOT = 4           # routed half-expert slots per core
P = 128
KH = H // P         # 16 contraction chunks over H
MI = I // P         # 11 I chunks (routed)
MSI = SI // P       # 22 I chunks (shared)
MH = H // P         # 16 output H chunks
TS = 4096 // N_CORES  # 512 tokens per core for the shared expert
BF16 = ml_dtypes.bfloat16

_COMPILED = {}  # caps tuple -> nc
_LAST = {}      # debug/profiling handle for test.py


def _gate_host(hs, gate_weight, bias):
    """numpy replica of reference._gate (verified bit-identical selection)."""
    T = hs.shape[0]
    logits = hs @ gate_weight.T                       # [T, E] fp32
    scores = 1.0 / (1.0 + np.exp(-logits))
    sfc = scores + bias[None, :]
    gs = sfc.reshape(T, G, E // G)
    gsort = np.sort(gs, axis=-1)
    group_scores = gsort[..., -1] + gsort[..., -2]
    group_idx = np.argsort(-group_scores, axis=-1, kind="stable")[:, :TOPK_GROUP]
    gmask = np.zeros((T, G), bool)
    gmask[np.arange(T)[:, None], group_idx] = True
    smask = np.repeat(gmask, E // G, axis=1)
    tmp = np.where(smask, sfc, 0.0)
    topk_idx = np.argsort(-tmp, axis=-1, kind="stable")[:, :TOP_K]
    topk_w = np.take_along_axis(scores, topk_idx, axis=1)
    topk_w = topk_w / (topk_w.sum(-1, keepdims=True) + 1e-20) * SCALE_FACTOR
    return topk_idx.astype(np.int32), topk_w.astype(np.float32)


def _build(caps):
    """Build + compile the SPMD Bass program.

    caps : per routed slot capacity (tokens, each = 2*w with w % 4 == 0)
    """
    import concourse.mybir as mybir
    import concourse.tile as tile
    from concourse import bacc

    bf = mybir.dt.bfloat16
    f32 = mybir.dt.float32
    AF = mybir.ActivationFunctionType

    C_tot = sum(caps)
    slot_base = [sum(caps[:s]) for s in range(len(caps))]
    ws = []
    for cap in caps:
        assert cap % 8 == 0
        ws.append(cap // 2)

    nc = bacc.Bacc("TRN2", target_bir_lowering=False, debug=False,
                   num_devices=N_CORES)
    # x is pre-arranged on the host into the SBUF tile layout
    # [128 partitions, KH * cap] (k-major, token minor) so every x load is
    # one fully-contiguous per-partition stream at full HBM rate
    xs = nc.dram_tensor("xs", [P, KH * TS], bf, kind="ExternalInput")
    xg = nc.dram_tensor("xg", [P, KH * C_tot], bf, kind="ExternalInput")
    # weight panels are pre-tiled on the host to the exact SBUF tile layout
    # [tile_idx, partition, ko*128+c] so every load is a contiguous
    # per-partition stream
    wg = nc.dram_tensor("wg", [NSLOT * MI, P, KH * P], bf,
                        kind="ExternalInput")
    wu = nc.dram_tensor("wu", [NSLOT * MI, P, KH * P], bf,
                        kind="ExternalInput")
    wd = nc.dram_tensor("wd", [NSLOT * MH, P, MI * P], bf,
                        kind="ExternalInput")
    sg = nc.dram_tensor("sg", [MSI, P, KH * P], bf, kind="ExternalInput")
    su = nc.dram_tensor("su", [MSI, P, KH * P], bf, kind="ExternalInput")
    sd = nc.dram_tensor("sd", [MH, P, MSI * P], bf, kind="ExternalInput")
    ye = nc.dram_tensor("ye", [H, C_tot], bf, kind="ExternalOutput")
    ys = nc.dram_tensor("ys", [H, TS], bf, kind="ExternalOutput")

    with tile.TileContext(nc) as tc:
        with (
            tc.tile_pool(name="xb", bufs=2) as xbp,    # [128,16,cap] x blocks
            tc.tile_pool(name="x0", bufs=8) as x0p,    # [128,2,cap] slot0 x
            tc.tile_pool(name="wp", bufs=8) as wp,     # [128,16,128] g/u cols
            tc.tile_pool(name="wq", bufs=16) as wqp,   # [128,4,128] warmup quarters
            tc.tile_pool(name="wdp", bufs=4) as wdp,   # [128,11,128] down
            tc.tile_pool(name="sdp", bufs=3) as sdp,   # [128,22,128] sh down
            tc.tile_pool(name="itp", bufs=27) as itp,  # [128,512] bf16 inter
            tc.tile_pool(name="tmp", bufs=4) as tmp,   # silu temp
            tc.tile_pool(name="otp", bufs=6) as otp,   # [128,<=1024] bf16 out
            tc.tile_pool(name="ps", bufs=8, space="PSUM") as psp,
        ):
            # ---------------- section order: routed slots 0..3, shared last.
            # Routed gate/up chains consume weights at ~94 GB/s (each tile
            # feeds 2*w columns) vs ~147 GB/s for shared, so starting with
            # a routed slot lets the DMA pipeline fill without stalling the
            # PE.  Each section's x tiles are drip-fed from inside the
            # previous section's down loop (one chunk per M iteration) so
            # the in-order scalar queue interleaves outputs and prefetch.
            def load_gu(pool, tagname, dram, row, nm, split):
                if split:
                    parts = []
                    for q in range(4):
                        t = wqp.tile([P, 4, P], bf, name=f"{nm}_{q}",
                                     tag="wq")
                        nc.sync.dma_start(
                            t[:],
                            dram[row].rearrange(
                                "p (ko c) -> p ko c", c=P)[:, 4 * q:4 * q + 4, :])
                        parts.append(t)
                    return parts
                t = pool.tile([P, KH, P], bf, name=nm, tag=tagname)
                nc.sync.dma_start(
                    t[:], dram[row].rearrange("p (ko c) -> p ko c", c=P))
                return t

            def gu_slice(t, k):
                if isinstance(t, list):
                    return t[k // 4][:, k % 4, :]
                return t[:, k, :]

            x_blocks = {}  # s -> big tile [P, 16, cap] ("sh" for shared)

            def emit_xg_block(s):
                # one contiguous mega-DMA loads all 16 contraction chunks
                # of this slot's gathered tokens
                cap = caps[s]
                off = KH * slot_base[s]
                t = xbp.tile([P, KH, cap], bf, name=f"xgb{s}", tag="xb")
                nc.scalar.dma_start(
                    t[:], xg[:, off:off + KH * cap].rearrange(
                        "p (k c) -> p k c", c=cap))
                x_blocks[s] = t

            def emit_xs_block():
                t = xbp.tile([P, KH, TS], bf, name="xsb", tag="xb")
                nc.gpsimd.dma_start(
                    t[:], xs.ap().rearrange("p (k c) -> p k c", c=TS))
                x_blocks["sh"] = t

            def emit_routed(s, first, prefetch):
                w = ws[s]
                cap = caps[s]
                b0 = slot_base[s]
                inter = {}
                if first:
                    # 8-chain k-outer warmup over m0+m1 x gate/up x j: each
                    # arriving x chunk feeds 8 matmuls so the PE keeps pace
                    # with the HBM feed during the first pass.
                    pst = {}
                    for mm_ in (0, 1):
                        for gu in ("g", "u"):
                            for j in range(2):
                                pst[(mm_, gu, j)] = psp.tile(
                                    [P, TS], f32, name=f"pw{mm_}{gu}{j}",
                                    tag="ps")
                    for k in range(KH):
                        for mm_ in (0, 1):
                            for gu in ("g", "u"):
                                for j in range(2):
                                    nc.tensor.matmul(
                                        pst[(mm_, gu, j)][:, :w],
                                        gu_slice(first[(mm_, gu)], k),
                                        x_slice(s, k)[:, j * w:(j + 1) * w],
                                        start=(k == 0), stop=(k == KH - 1))
                    for mm_ in (0, 1):
                        for j in range(2):
                            st = tmp.tile([P, TS], bf,
                                          name=f"st{s}_{mm_}{j}", tag="tmp")
                            nc.scalar.activation(
                                st[:, :w], pst[(mm_, "g", j)][:, :w],
                                AF.Silu)
                            it = itp.tile([P, TS], bf,
                                          name=f"it{s}_{mm_}{j}", tag="it")
                            nc.vector.tensor_mul(
                                it[:, :w], st[:, :w],
                                pst[(mm_, "u", j)][:, :w])
                            inter[(mm_, j)] = it
                for m in range(2 if first else 0, MI):
                    if m == 5:
                        # prefetch the next section's x block while this
                        # slot still has ~60us of gate/up left
                        if prefetch == "sh":
                            emit_xs_block()
                        else:
                            emit_xg_block(prefetch)
                    wgt = load_gu(wp, "wp", wg, s * MI + m,
                                  f"wgt{s}_{m}", False)
                    wut = load_gu(wp, "wp", wu, s * MI + m,
                                  f"wut{s}_{m}", False)
                    for j in range(2):
                        psg = psp.tile([P, TS], f32, name=f"psg{s}_{m}{j}",
                                       tag="ps")
                        for k in range(KH):
                            nc.tensor.matmul(
                                psg[:, :w], gu_slice(wgt, k),
                                x_slice(s, k)[:, j * w:(j + 1) * w],
                                start=(k == 0), stop=(k == KH - 1))
                        st = tmp.tile([P, TS], bf, name=f"st{s}_{m}{j}",
                                      tag="tmp")
                        nc.scalar.activation(st[:, :w], psg[:, :w], AF.Silu)
                        psu = psp.tile([P, TS], f32, name=f"psu{s}_{m}{j}",
                                       tag="ps")
                        for k in range(KH):
                            nc.tensor.matmul(
                                psu[:, :w], gu_slice(wut, k),
                                x_slice(s, k)[:, j * w:(j + 1) * w],
                                start=(k == 0), stop=(k == KH - 1))
                        it = itp.tile([P, TS], bf, name=f"it{s}_{m}{j}",
                                      tag="it")
                        nc.vector.tensor_mul(it[:, :w], st[:, :w],
                                             psu[:, :w])
                        inter[(m, j)] = it
                for M in range(MH):
                    wdt = wdp.tile([P, MI, P], bf, name=f"wdt{s}_{M}",
                                   tag="wdt")
                    nc.sync.dma_start(
                        wdt[:], wd[s * MH + M].rearrange(
                            "p (ko c) -> p ko c", c=P))
                    ot = otp.tile([P, 2 * w], bf, name=f"ot{s}_{M}", tag="ot")
                    for j in range(2):
                        psy = psp.tile([P, TS], f32, name=f"psy{s}_{M}{j}",
                                       tag="ps")
                        for K in range(MI):
                            nc.tensor.matmul(
                                psy[:, :w], wdt[:, K, :],
                                inter[(K, j)][:, :w],
                                start=(K == 0), stop=(K == MI - 1))
                        nc.vector.tensor_copy(
                            ot[:, j * w:(j + 1) * w], psy[:, :w])
                    nc.scalar.dma_start(
                        ye[M * P:(M + 1) * P, b0:b0 + cap], ot[:])

            def emit_shared():
                sint = []
                for m in range(MSI):
                    sgt = load_gu(wp, "wp", sg, m, f"sgt{m}", False)
                    sut = load_gu(wp, "wp", su, m, f"sut{m}", False)
                    psg = psp.tile([P, TS], f32, name=f"psgs{m}", tag="ps")
                    for k in range(KH):
                        nc.tensor.matmul(psg[:], sgt[:, k, :],
                                         x_blocks["sh"][:, k, :],
                                         start=(k == 0), stop=(k == KH - 1))
                    st = tmp.tile([P, TS], bf, name=f"sts{m}", tag="tmp")
                    nc.scalar.activation(st[:], psg[:], AF.Silu)
                    psu = psp.tile([P, TS], f32, name=f"psus{m}", tag="ps")
                    for k in range(KH):
                        nc.tensor.matmul(psu[:], sut[:, k, :],
                                         x_blocks["sh"][:, k, :],
                                         start=(k == 0), stop=(k == KH - 1))
                    it = itp.tile([P, TS], bf, name=f"si{m}", tag="it")
                    nc.vector.tensor_mul(it[:], st[:], psu[:])
                    sint.append(it)
                for M in range(MH):
                    sdt = sdp.tile([P, MSI, P], bf, name=f"sdt{M}", tag="sdt")
                    nc.sync.dma_start(
                        sdt[:], sd[M].rearrange("p (ko c) -> p ko c", c=P))
                    psy = psp.tile([P, TS], f32, name=f"psys{M}", tag="ps")
                    for K in range(MSI):
                        nc.tensor.matmul(psy[:], sdt[:, K, :], sint[K][:],
                                         start=(K == 0), stop=(K == MSI - 1))
                    ot = otp.tile([P, TS], bf, name=f"ots{M}", tag="ot")
                    nc.vector.tensor_copy(ot[:], psy[:])
                    nc.scalar.dma_start(ys[M * P:(M + 1) * P, :], ot[:])

            # slot0 startup: m0 weight quarters on sync, x in 4 block-DMAs
            # on scalar, interleaved so the k-outer warmup streams at the
            # DMA feed rate from the first landing block.
            wq0 = {}
            x0_blocks = []
            for q in range(4):
                for mm_ in (0, 1):
                    for dram, nm in ((wg, "g"), (wu, "u")):
                        t = wqp.tile([P, 4, P], bf, name=f"w{nm}{mm_}_{q}",
                                     tag="wq")
                        nc.sync.dma_start(
                            t[:],
                            dram[mm_].rearrange(
                                "p (ko c) -> p ko c",
                                c=P)[:, 4 * q:4 * q + 4, :])
                        wq0.setdefault((mm_, nm), []).append(t)
                for h in range(2):
                    t = x0p.tile([P, 2, caps[0]], bf,
                                 name=f"x0b{2 * q + h}", tag="x0")
                    kk = 4 * q + 2 * h
                    nc.scalar.dma_start(
                        t[:], xg[:, kk * caps[0]:(kk + 2) * caps[0]]
                        .rearrange("p (k c) -> p k c", c=caps[0]))
                    x0_blocks.append(t)

            def x_slice(s, k):
                if s == 0:
                    return x0_blocks[k // 2][:, k % 2, :]
                return x_blocks[s][:, k, :]

            emit_routed(0, wq0, 1)
            emit_routed(1, False, 2)
            emit_routed(2, False, 3)
            emit_routed(3, False, "sh")
            emit_shared()

    nc.compile()
    return nc


def _get_compiled(caps):
    key = tuple(caps)
    if key not in _COMPILED:
        _COMPILED[key] = _build(caps)
    return _COMPILED[key]


def _tile_gu(wmat, mi):  # [mi*P, H] -> [mi, P, KH*P] : (m, c_i, ko_h*P + p_i)
    return np.ascontiguousarray(
        wmat.reshape(mi, P, KH, P).transpose(0, 3, 2, 1)
    ).reshape(mi, P, KH * P).astype(BF16)


def _tile_dn(wmat, mi):  # [H, mi*P] -> [MH, P, mi*P]
    return np.ascontiguousarray(
        wmat.reshape(MH, P, mi, P).transpose(0, 3, 2, 1)
    ).reshape(MH, P, mi * P).astype(BF16)


def kernel(hidden_states, gate_weight, e_score_correction_bias,
           gate_proj, up_proj, down_proj,
           shared_gate_w, shared_up_w, shared_down_w):
    from concourse.bass_utils import run_bass_kernel_spmd

    hs = np.asarray(hidden_states, dtype=np.float32)
    B, S, Hh = hs.shape
    assert Hh == H
    hsf = np.ascontiguousarray(hs.reshape(-1, H))
    T = hsf.shape[0]
    assert T == N_CORES * TS
    gate_weight = np.asarray(gate_weight, np.float32)
    bias = np.asarray(e_score_correction_bias, np.float32)
    gate_proj = np.asarray(gate_proj, np.float32)
    up_proj = np.asarray(up_proj, np.float32)
    down_proj = np.asarray(down_proj, np.float32)
    shared_gate_w = np.asarray(shared_gate_w, np.float32)
    shared_up_w = np.asarray(shared_up_w, np.float32)
    shared_down_w = np.asarray(shared_down_w, np.float32)

    # ---- routing on host ----
    topk_idx, topk_w = _gate_host(hsf, gate_weight, bias)
    comb = np.zeros((T, E), np.float32)
    np.add.at(comb, (np.arange(T)[:, None], topk_idx), topk_w)
    sel = np.zeros((T, E), bool)
    sel[np.arange(T)[:, None], topk_idx] = True
    idx_e = [np.nonzero(sel[:, e])[0] for e in range(E)]

    # split each expert's tokens in half; rank the 32 pieces by size and
    # deal them into NSLOT slots of 8 cores so per-slot capacity (uniform
    # across cores under SPMD) hugs the max piece of its rank
    order = np.argsort([-len(ix) for ix in idx_e], kind="stable")
    caps, assign = [], []
    for r in range(NSLOT):
        grp = []
        for e in order[4 * r:4 * r + 4]:
            n = len(idx_e[e])
            h1 = (n + 1) // 2
            grp.append((int(e), idx_e[e][:h1]))
            grp.append((int(e), idx_e[e][h1:]))
        cap = max(8, -(-max(len(p[1]) for p in grp) // 8) * 8)
        caps.append(cap)
        assign.append(grp)
    slot_base = [sum(caps[:s]) for s in range(NSLOT)]
    C_tot = sum(caps)

    # ---- host-side dispatch (shard + transpose + bf16 cast) ----
    xsT = np.ascontiguousarray(hsf.T).astype(BF16)          # [H, T]

    gu_g = {e: _tile_gu(gate_proj[e], MI) for e in range(E)}
    gu_u = {e: _tile_gu(up_proj[e], MI) for e in range(E)}
    dn = {e: _tile_dn(down_proj[e], MI) for e in range(E)}
    sg_p = _tile_gu(shared_gate_w, MSI)
    su_p = _tile_gu(shared_up_w, MSI)
    sd_p = _tile_dn(shared_down_w, MSI)

    def x_tilelayout(xpiece, cap):
        # [H, n] -> [128, KH*cap] (k-major, token minor), zero-padded
        n = xpiece.shape[1]
        out = np.zeros((P, KH, cap), BF16)
        out[:, :, :n] = xpiece.reshape(KH, P, n).transpose(1, 0, 2)
        return out.reshape(P, KH * cap)

    in_maps = []
    for c in range(N_CORES):
        xg_c = np.concatenate(
            [x_tilelayout(xsT[:, assign[r][c][1]], caps[r])
             for r in range(NSLOT)], axis=1)
        wg_c = np.concatenate([gu_g[assign[r][c][0]] for r in range(NSLOT)])
        wu_c = np.concatenate([gu_u[assign[r][c][0]] for r in range(NSLOT)])
        wd_c = np.concatenate([dn[assign[r][c][0]] for r in range(NSLOT)])
        xs_c = x_tilelayout(xsT[:, c * TS:(c + 1) * TS], TS)
        in_maps.append({
            "xs": xs_c, "xg": xg_c,
            "wg": wg_c, "wu": wu_c, "wd": wd_c,
            "sg": sg_p, "su": su_p, "sd": sd_p,
        })

    nc = _get_compiled(caps)
    results = run_bass_kernel_spmd(nc, in_maps, core_ids=list(range(N_CORES)))

    _LAST.clear()
    _LAST.update(nc=nc, in_maps=in_maps, results=results, caps=caps,
                 assign=assign)

    # ---- host-side combine ----
    outT = np.empty((H, T), np.float32)
    for c in range(N_CORES):
        outT[:, c * TS:(c + 1) * TS] = results.results[c]["ys"]
    for c in range(N_CORES):
        ye = results.results[c]["ye"].astype(np.float32)
        for r in range(NSLOT):
            e, idx = assign[r][c]
            if len(idx) == 0:
                continue
            b0 = slot_base[r]
            we = comb[idx, e]
            outT[:, idx] += ye[:, b0:b0 + len(idx)] * we[None, :]

    return np.ascontiguousarray(outT.T).reshape(B, S, H).astype(np.float32)
